# revision 2
# baseline (speedup 1.0000x reference)
"""GroupedQueryAttention kernel v3 for 8 Trainium2 NeuronCores.

Sharding: tensor-parallel over KV groups (core c owns group c = 4 query
heads x 64); x replicated; host sums partial outputs in f32.

v3 vs v2 (281us): host-side partition-major layouts (all DMAs become
contiguous [128, N] copies), one flat software pipeline across all
(chunk, pair, sk) items so the next pair's scores are always emitted
ahead of the previous pair's epilogue (keeps ScalarE fed at pair and
chunk boundaries), ldweights=False weight reuse on repeated-stationary
matmuls, and a tail o-proj that alternates PSUM evacuation between the
Vector and Scalar engines.

Layouts per core (S=2048, D=2048, 4 heads of 64):
  xT_sb  [128, 16, 2048] bf16   x^T k-tiles (host partition-major)
  qT_sb  [128, 2, 2048]  bf16   Q^T; head h -> partitions 64*(h%2), slot h//2
  kT_sb  [128, 2048]     bf16   K^T duplicated on both partition halves
  v1_sb  [128, 16, 65]   bf16   [V | ones] natural layout per sk tile
  oT_sb  [128, 2, 2048]  bf16   normalized attention out (same map as qT)
  out    [2048, 2048]    f32    partial output, host-summed
"""

import numpy as np

S = 2048
D = 2048
N_CORES = 8
HD = 64
HPG = 4
QDIM = HPG * HD           # 256
SCALE = 1.0 / 8.0         # 1/sqrt(HD)
SQC = 512                 # seq chunk (psum bank width in f32)
NCH = S // SQC            # 4
T = S // 128              # 16 sk tiles
KO = D // 128             # 16 contraction tiles
QT = QDIM // 128          # 2 q partition tiles (= head pairs)

_compiled = {}


def _noldw(bi):
    """Mark a matmul as non-self-loading (reuses the PE array weights
    loaded by the immediately preceding matmul on the Tensor engine)."""
    bi.ins.ldweights = False
    return bi


def build_gqa(debug=False):
    import concourse.tile as tile
    from concourse import bacc, mybir
    from concourse.masks import make_identity
    from contextlib import ExitStack

    f32 = mybir.dt.float32
    bf16 = mybir.dt.bfloat16
    EXP = mybir.ActivationFunctionType.Exp

    nc = bacc.Bacc(None, target_bir_lowering=False, debug=debug)
    # host provides partition-major layouts: [p, ...] with p the SBUF partition
    xTp = nc.declare_dram_parameter("xTp", [128, KO * S], bf16, isOutput=False)
    wqp = nc.declare_dram_parameter("wqp", [128, KO * QDIM], bf16, isOutput=False)
    wkvp = nc.declare_dram_parameter("wkvp", [128, KO * 2 * HD], bf16, isOutput=False)
    wop = nc.declare_dram_parameter("wop", [128, QT * D], bf16, isOutput=False)
    out = nc.declare_dram_parameter("out", [S, D], f32, isOutput=True)

    with tile.TileContext(nc) as tc, ExitStack() as ctx:
        const = ctx.enter_context(tc.tile_pool(name="const", bufs=1))
        persist = ctx.enter_context(tc.tile_pool(name="persist", bufs=1))

        ident = const.tile([128, 128], bf16)
        make_identity(nc, ident)
        ones_bf = const.tile([1, HD], bf16)
        nc.vector.memset(ones_bf, 1.0)
        bias_exp = const.tile([128, 1], f32)
        nc.vector.memset(bias_exp, -8.0)

        xT_sb = persist.tile([128, KO, S], bf16)
        qT_sb = persist.tile([128, QT, S], bf16)
        kT_sb = persist.tile([128, S], bf16)
        v1_sb = persist.tile([128, T, HD + 1], bf16)
        oT_sb = persist.tile([128, QT, S], bf16)
        wkv_sb = persist.tile([128, KO, 2 * HD], bf16)
        wq_sb = persist.tile([128, KO, QDIM], bf16)
        wo_sb = persist.tile([128, QT, D], bf16)

        nc.vector.memset(v1_sb[:, :, HD:HD + 1], 1.0)

        # ---------------- input DMAs (contiguous per partition) ----------
        nc.sync.dma_start(out=wkv_sb, in_=wkvp[:].rearrange("p (ko m) -> p ko m", ko=KO))
        nc.sync.dma_start(out=wq_sb, in_=wqp[:].rearrange("p (ko m) -> p ko m", ko=KO))
        for ko in range(KO):
            nc.sync.dma_start(
                out=xT_sb[:, ko, :], in_=xTp[:, ko * S:(ko + 1) * S])
        nc.sync.dma_start(out=wo_sb, in_=wop[:].rearrange("p (qt m) -> p qt m", qt=QT))

        # ---------------- prologue: projections ----------------
        pev = ctx.enter_context(tc.tile_pool(name="pev", bufs=2))
        with tc.tile_pool(name="ppool", bufs=8, space="PSUM") as pp:
            kv_ps = [pp.tile([128, SQC], f32, name=f"kv{ch}", tag="pp")
                     for ch in range(NCH)]
            q0_ps = [pp.tile([128, SQC], f32, name=f"q0{ch}", tag="pp")
                     for ch in range(NCH)]
            for ko in range(KO):
                for ch in range(NCH):
                    cs = slice(ch * SQC, (ch + 1) * SQC)
                    bi = nc.tensor.matmul(
                        kv_ps[ch], wkv_sb[:, ko, :], xT_sb[:, ko, cs],
                        start=(ko == 0), stop=(ko == KO - 1))
                    if ch:
                        _noldw(bi)
                for ch in range(NCH):
                    cs = slice(ch * SQC, (ch + 1) * SQC)
                    bi = nc.tensor.matmul(
                        q0_ps[ch], wq_sb[:, ko, 0:128], xT_sb[:, ko, cs],
                        start=(ko == 0), stop=(ko == KO - 1))
                    if ch:
                        _noldw(bi)

            vT_tmp = pev.tile([64, S], bf16, name="vT_tmp", tag="vt")
            for ch in range(NCH):
                cs = slice(ch * SQC, (ch + 1) * SQC)
                nc.vector.tensor_copy(out=kT_sb[0:64, cs], in_=kv_ps[ch][0:64, :])
                nc.vector.tensor_copy(out=kT_sb[64:128, cs], in_=kv_ps[ch][0:64, :])
                nc.vector.tensor_copy(out=vT_tmp[:, cs], in_=kv_ps[ch][64:128, :])

            q1_ps = [pp.tile([128, SQC], f32, name=f"q1{ch}", tag="pp")
                     for ch in range(NCH)]
            for ko in range(KO):
                for ch in range(NCH):
                    cs = slice(ch * SQC, (ch + 1) * SQC)
                    bi = nc.tensor.matmul(
                        q1_ps[ch], wq_sb[:, ko, 128:256], xT_sb[:, ko, cs],
                        start=(ko == 0), stop=(ko == KO - 1))
                    if ch:
                        _noldw(bi)
            for ch in range(NCH):
                cs = slice(ch * SQC, (ch + 1) * SQC)
                nc.vector.tensor_copy(out=qT_sb[:, 0, cs], in_=q0_ps[ch])
            for ch in range(NCH):
                cs = slice(ch * SQC, (ch + 1) * SQC)
                nc.vector.tensor_copy(out=qT_sb[:, 1, cs], in_=q1_ps[ch])

        # V transposes into natural layout (separate psum scope)
        with tc.tile_pool(name="vtps", bufs=2, space="PSUM") as vtps:
            for t in range(T):
                pt = vtps.tile([128, HD], bf16, name="pt")
                nc.tensor.transpose(
                    pt, vT_tmp[:, t * 128:(t + 1) * 128], ident[0:64, 0:64])
                nc.vector.tensor_copy(out=v1_sb[:, t, 0:HD], in_=pt)

        # ---------------- phase 2: attention + o-proj, one flat pipeline --
        scps = ctx.enter_context(tc.tile_pool(name="scps", bufs=2, space="PSUM"))
        avps = ctx.enter_context(tc.tile_pool(name="avps", bufs=2, space="PSUM"))
        mips = ctx.enter_context(tc.tile_pool(name="mips", bufs=2, space="PSUM"))
        eps = ctx.enter_context(tc.tile_pool(name="eps", bufs=8))
        p2ev = ctx.enter_context(tc.tile_pool(name="p2ev", bufs=4))
        ypool = ctx.enter_context(tc.tile_pool(name="ypool", bufs=4))

        def emit_oproj_task(t, och, tail_idx=-1):
            """o-proj for seq tile t, one output column chunk."""
            ns = slice(och * SQC, (och + 1) * SQC)
            py = mips.tile([128, SQC], f32, name="py", tag="mip")
            for qt in range(QT):
                nc.tensor.matmul(
                    py, oT_sb[:, qt, t * 128:(t + 1) * 128], wo_sb[:, qt, ns],
                    start=(qt == 0), stop=(qt == QT - 1))
            y_sb = ypool.tile([128, SQC], f32, name="y_sb")
            if tail_idx >= 0 and tail_idx % 2 == 1:
                nc.scalar.copy(out=y_sb, in_=py)
            else:
                nc.vector.tensor_copy(out=y_sb, in_=py)
            nc.sync.dma_start(
                out=out[:].rearrange("(t p) n -> p t n", p=128)[:, t, ns],
                in_=y_sb)

        def evacuate_av(av):
            """Free the AV psum banks fast: pull denominator + raw O to SBUF.
            Normalization happens later, off the AV-accumulator critical path."""
            den, orw = [], []
            for hh in range(2):
                den.append(p2ev.tile([1, SQC], f32, name=f"den{hh}", tag=f"den{hh}"))
                nc.vector.tensor_copy(out=den[hh], in_=av[hh][HD:HD + 1, :])
                orw.append(p2ev.tile([HD, SQC], bf16, name=f"orw{hh}", tag=f"orw{hh}"))
                with nc.allow_low_precision(reason="bf16 attn out"):
                    nc.vector.tensor_copy(out=orw[hh], in_=av[hh][0:HD, :])
            return den, orw

        def make_epilogue(ch, qt, den, orw):
            cs = slice(ch * SQC, (ch + 1) * SQC)

            def epi():
                for hh in range(2):
                    rf = p2ev.tile([1, SQC], f32, name=f"rf{hh}", tag=f"rf{hh}")
                    with nc.allow_low_precision(reason="softmax recip ~51ulp"):
                        nc.vector.reciprocal_approx_fast(out=rf, in_=den[hh])
                    rec = p2ev.tile([1, SQC], bf16, name=f"rec{hh}", tag=f"rec{hh}")
                    with nc.allow_low_precision(reason="bf16 recip bcast"):
                        nc.vector.tensor_copy(out=rec, in_=rf)
                    bc = mips.tile([128, SQC], f32, name="bc", tag="mip")
                    nc.tensor.matmul(
                        bc[0:HD, :], ones_bf, rec, start=True, stop=True)
                    bc_sb = p2ev.tile([HD, SQC], bf16, name=f"bcs{hh}", tag=f"bcs{hh}")
                    with nc.allow_low_precision(reason="bf16 recip bcast"):
                        nc.vector.tensor_copy(out=bc_sb, in_=bc[0:HD, :])
                    with nc.allow_low_precision(reason="bf16 attn out"):
                        nc.vector.tensor_mul(
                            out=oT_sb[64 * hh:64 * hh + 64, qt, cs],
                            in0=orw[hh], in1=bc_sb)
            return epi

        items = [(ch, qt, sk)
                 for ch in range(NCH) for qt in range(QT) for sk in range(T)]
        sc_tiles = {}

        def emit_scores(idx):
            ch, qt, sk = items[idx]
            cs = slice(ch * SQC, (ch + 1) * SQC)
            sc = scps.tile([128, 2, SQC], f32, name="sc", tag="sc")
            for hh in range(2):
                hp = 64 * hh
                nc.tensor.matmul(
                    sc[:, hh, :],
                    kT_sb[hp:hp + 64, sk * 128:(sk + 1) * 128],
                    qT_sb[hp:hp + 64, qt, cs],
                    start=True, stop=True)
            sc_tiles[idx] = sc

        pending_epi = None
        fillers = []
        av = None
        for idx, (ch, qt, sk) in enumerate(items):
            if sk == 0:
                if idx == 0:
                    emit_scores(0)
                    emit_scores(1)
                av = [avps.tile([HD + 1, SQC], f32, name=f"av{hh}", tag="av")
                      for hh in range(2)]
                if qt == 0:  # new chunk: queue o-proj of previous chunk
                    if ch > 0:
                        base = (ch - 1) * (SQC // 128)
                        fillers = [
                            (lambda t=base + tt, oc=oc: emit_oproj_task(t, oc))
                            for tt in range(SQC // 128) for oc in range(NCH)
                        ]
            e_sb = eps.tile([128, 2, SQC], bf16, name="e_sb")
            nc.scalar.activation(
                out=e_sb, in_=sc_tiles.pop(idx),
                func=EXP, bias=bias_exp, scale=1.0)
            if idx + 2 < len(items):
                emit_scores(idx + 2)
            if pending_epi is not None:
                pending_epi()
                pending_epi = None
            for hh in range(2):
                bi = nc.tensor.matmul(
                    av[hh][:, :], v1_sb[:, sk, :], e_sb[:, hh, :],
                    start=(sk == 0), stop=(sk == T - 1))
                if hh:
                    _noldw(bi)
            if sk % 2 == 1 and fillers:
                fillers.pop(0)()
            if sk == T - 1:
                den, orw = evacuate_av(av)
                pending_epi = make_epilogue(ch, qt, den, orw)
        pending_epi()
        for f in fillers:
            f()
        # tail: o-proj of last chunk
        base = (NCH - 1) * (SQC // 128)
        i = 0
        for tt in range(SQC // 128):
            for oc in range(NCH):
                emit_oproj_task(base + tt, oc, tail_idx=i)
                i += 1

    nc.compile()
    return nc


def _get_nc():
    if "nc" not in _compiled:
        _compiled["nc"] = build_gqa()
    return _compiled["nc"]


def _pm(a):
    """[KO*128, M] -> partition-major [128, KO*M] (row p holds all ko chunks)."""
    ko = a.shape[0] // 128
    return np.ascontiguousarray(
        a.reshape(ko, 128, a.shape[1]).transpose(1, 0, 2).reshape(128, -1))


def _shard_inputs(x, w_q, w_k, w_v, w_o):
    import ml_dtypes

    bf = ml_dtypes.bfloat16
    x = np.asarray(x, dtype=np.float32)
    w_q = np.asarray(w_q, dtype=np.float32)
    w_k = np.asarray(w_k, dtype=np.float32)
    w_v = np.asarray(w_v, dtype=np.float32)
    w_o = np.asarray(w_o, dtype=np.float32)
    xT = np.ascontiguousarray(x.reshape(S, D).T)
    xTp = _pm(xT).astype(bf)
    in_maps = []
    for c in range(N_CORES):
        wkv = np.concatenate(
            [w_k[:, c * HD:(c + 1) * HD], w_v[:, c * HD:(c + 1) * HD]], axis=1)
        in_maps.append({
            "xTp": xTp,
            "wqp": _pm(w_q[:, c * QDIM:(c + 1) * QDIM] * np.float32(SCALE)).astype(bf),
            "wkvp": _pm(wkv).astype(bf),
            "wop": _pm(w_o[c * QDIM:(c + 1) * QDIM, :]).astype(bf),
        })
    return in_maps


def kernel(x, w_q, w_k, w_v, w_o):
    from concourse.bass_utils import run_bass_kernel_spmd

    nc = _get_nc()
    in_maps = _shard_inputs(x, w_q, w_k, w_v, w_o)
    res = run_bass_kernel_spmd(nc, in_maps, list(range(N_CORES)))
    acc = np.zeros((S, D), dtype=np.float64)
    for r in res.results:
        acc += r["out"].astype(np.float64)
    return acc.astype(np.float32).reshape(1, S, D)
